# revision 27
# baseline (speedup 1.0000x reference)
"""CenterLoss kernel for Trainium2, data-parallel across 8 NeuronCores.

Math: the reference masks the full [B, C] squared-distance matrix with
one_hot(labels) and clamps to [1e-12, 1e12] before summing.  The mask keeps
only distmat[i, labels[i]]; every other entry becomes clip(0) = 1e-12.  The
kept entries are ~1024 (chi-square-like, 512 dof), so the clamp never binds
on them and the loss reduces to

    loss = ( sum_i ||x_i - c_{l_i}||^2 + B*(C-1)*1e-12 ) / B

Per core (B/8 = 2048 rows), raw bass pipeline, p-major row layout
(shard row 16*p + r lives at partition p, chunk r; r in [0,16)):
  sync   : 4x 1MB x loads (HWDGE, 8KB/partition contiguous descriptors)
  gpsimd : label load + 16x indirect_dma_start (row r: out[p,:] =
           centers[labels[16p+r], :])
  vector : diff = x - g per chunk            [128, 512]
  scalar : Square activation + row-accumulate -> acc[:, r], then acc store
Host sums the 8 x [128, 16] partials in f64 and adds the clamp constant.
"""

import sys
from contextlib import ExitStack

import numpy as np

try:
    import concourse.bass  # noqa: F401
except ImportError:
    sys.path.insert(0, "/opt/trn_rl_repo")

import concourse.bass as bass
import concourse.mybir as mybir
from concourse.bacc import Bacc
from concourse.bass_utils import run_bass_kernel_spmd

B, C, D = 16384, 1000, 512
N_CORES = 8
B_SHARD = B // N_CORES  # 2048
P = 128
NCHUNK = B_SHARD // P  # 16 chunks, chunk r = rows {16p + r}
CLAMP_MIN = 1e-12
CLAMP_MAX = 1e12

_NC_CACHE = {}


def build_nc():
    nc = Bacc()
    f32 = mybir.dt.float32
    x_d = nc.declare_dram_parameter("x", [B_SHARD, D], f32, isOutput=False)
    lbl_d = nc.declare_dram_parameter(
        "labels", [P, NCHUNK], mybir.dt.int32, isOutput=False
    )
    cen_d = nc.declare_dram_parameter("centers", [C, D], f32, isOutput=False)
    out_d = nc.declare_dram_parameter("out", [P, NCHUNK], f32, isOutput=True)

    x_r = x_d.rearrange("(p r) d -> p r d", p=P)  # [128, 16, 512], contiguous per p

    with ExitStack() as ctx:
        x_sb = ctx.enter_context(nc.sbuf_tensor("x_sb", [P, NCHUNK, D], f32))
        g_sb = ctx.enter_context(nc.sbuf_tensor("g_sb", [P, NCHUNK, D], f32))
        diff_sb = ctx.enter_context(nc.sbuf_tensor("diff_sb", [P, 2, D], f32))
        sq_sb = ctx.enter_context(nc.sbuf_tensor("sq_sb", [P, D], f32))
        lbl_sb = ctx.enter_context(nc.sbuf_tensor("lbl_sb", [P, NCHUNK], mybir.dt.int32))
        acc_sb = ctx.enter_context(nc.sbuf_tensor("acc_sb", [P, NCHUNK], f32))

        block = ctx.enter_context(nc.Block())
        ls = ctx.enter_context(nc.semaphore("ls"))
        # x load groups: small leading groups so compute starts early,
        # 1MB bulk groups for descriptor efficiency
        XG = [(0, 2), (2, 4), (4, 8), (8, 12), (12, 16)]
        xs = [ctx.enter_context(nc.semaphore(f"xs{q}")) for q in range(len(XG))]
        gs = [ctx.enter_context(nc.semaphore(f"gs{r}")) for r in range(NCHUNK)]
        vs = ctx.enter_context(nc.semaphore("vs"))
        ss = ctx.enter_context(nc.semaphore("ss"))
        os_ = ctx.enter_context(nc.semaphore("os"))

        @block.sync
        def _(sync):
            # labels first: the gather stream (Q7 descriptor emission) is the
            # critical path and only needs this tiny tile
            sync.dma_start(out=lbl_sb[:], in_=lbl_d[:]).then_inc(ls, 16)
            for q, (a, b) in enumerate(XG):
                sync.dma_start(
                    out=x_sb[:, a:b, :], in_=x_r[:, a:b, :]
                ).then_inc(xs[q], 16)


        @block.gpsimd
        def _(gpsimd):
            gpsimd.wait_ge(ls, 16)
            for r in range(NCHUNK):
                gpsimd.indirect_dma_start(
                    out=g_sb[:, r, :],
                    out_offset=None,
                    in_=cen_d[:],
                    in_offset=bass.IndirectOffsetOnAxis(
                        ap=lbl_sb[:, r : r + 1], axis=0
                    ),
                ).then_inc(gs[r], 16)

        XQ = [q for q, (a, b) in enumerate(XG) for _ in range(b - a)]

        @block.vector
        def _(vector):
            for r in range(NCHUNK):
                vector.wait_ge(xs[XQ[r]], 16)
                vector.wait_ge(gs[r], 16)
                if r >= 2:
                    vector.wait_ge(ss, r - 1)  # WAR: scalar done with diff slot
                vector.tensor_tensor(
                    out=diff_sb[:, r % 2, :],
                    in0=x_sb[:, r, :],
                    in1=g_sb[:, r, :],
                    op=mybir.AluOpType.subtract,
                ).then_inc(vs, 1)

        @block.scalar
        def _(scalar):
            for r in range(NCHUNK):
                scalar.wait_ge(vs, r + 1)
                scalar.activation(
                    out=sq_sb[:, :],
                    in_=diff_sb[:, r % 2, :],
                    func=mybir.ActivationFunctionType.Square,
                    accum_out=acc_sb[:, r : r + 1],
                ).then_inc(ss, 1)
                if r == NCHUNK - 3:
                    # early store of the first 14 columns hides most of the
                    # final DMA's completion receipt behind the last chunks
                    scalar.dma_start(
                        out=out_d[:, : NCHUNK - 2], in_=acc_sb[:, : NCHUNK - 2]
                    ).then_inc(os_, 16)
            scalar.dma_start(
                out=out_d[:, NCHUNK - 2 :], in_=acc_sb[:, NCHUNK - 2 :]
            ).then_inc(os_, 16)
            scalar.wait_ge(os_, 32)

    nc.finalize()
    return nc


def _get_nc():
    if "nc" not in _NC_CACHE:
        _NC_CACHE["nc"] = build_nc()
    return _NC_CACHE["nc"]


def kernel(x, labels, centers, _trace=False):
    x = np.asarray(x, dtype=np.float32)
    centers = np.asarray(centers, dtype=np.float32)
    labels_i = np.asarray(labels).astype(np.int32)

    in_maps = []
    for i in range(N_CORES):
        xs_ = np.ascontiguousarray(x[i * B_SHARD : (i + 1) * B_SHARD])
        ls_ = labels_i[i * B_SHARD : (i + 1) * B_SHARD]
        in_maps.append(
            {
                "x": xs_,
                # row 16p + r at [p, r]
                "labels": np.ascontiguousarray(ls_.reshape(P, NCHUNK)),
                "centers": centers,
            }
        )

    nc = _get_nc()
    res = run_bass_kernel_spmd(nc, in_maps, list(range(N_CORES)), trace=_trace)
    partials = np.stack([r["out"] for r in res.results])  # [8, 128, 16]
    total = np.sum(partials.astype(np.float64))
    total += B * (C - 1) * CLAMP_MIN
    loss = np.float32(total / B)
    if _trace:
        return np.asarray(loss), res
    return np.asarray(loss)


# revision 28
# speedup vs baseline: 1.2022x; 1.2022x over previous
"""CenterLoss kernel for Trainium2, data-parallel across 8 NeuronCores.

Math: the reference masks the full [B, C] squared-distance matrix with
one_hot(labels) and clamps to [1e-12, 1e12] before summing.  The mask keeps
only distmat[i, labels[i]]; every other entry becomes clip(0) = 1e-12.  The
kept entries are ~1024 (chi-square-like, 512 dof), so the clamp never binds
on them and the loss reduces to

    loss = ( sum_i ||x_i - c_{l_i}||^2 + B*(C-1)*1e-12 ) / B

Per core (B/8 = 2048 rows), raw bass pipeline, p-major row layout
(shard row 16*p + r lives at partition p, chunk r; r in [0,16)):
  sync   : 4x 1MB x loads (HWDGE, 8KB/partition contiguous descriptors)
  gpsimd : label load + 16x indirect_dma_start (row r: out[p,:] =
           centers[labels[16p+r], :])
  vector : diff = x - g per chunk            [128, 512]
  scalar : Square activation + row-accumulate -> acc[:, r], then acc store
Host sums the 8 x [128, 16] partials in f64 and adds the clamp constant.
"""

import sys
from contextlib import ExitStack

import numpy as np

try:
    import concourse.bass  # noqa: F401
except ImportError:
    sys.path.insert(0, "/opt/trn_rl_repo")

import concourse.bass as bass
import concourse.mybir as mybir
from concourse.bacc import Bacc
from concourse.bass_utils import run_bass_kernel_spmd

B, C, D = 16384, 1000, 512
N_CORES = 8
B_SHARD = B // N_CORES  # 2048
P = 128
NCHUNK = B_SHARD // P  # 16 chunks, chunk r = rows {16p + r}
NXD = 4  # x loads (4 chunks each)
CLAMP_MIN = 1e-12
CLAMP_MAX = 1e12

_NC_CACHE = {}


def build_nc():
    nc = Bacc()
    f32 = mybir.dt.float32
    x_d = nc.declare_dram_parameter("x", [B_SHARD, D], f32, isOutput=False)
    lbl_d = nc.declare_dram_parameter(
        "labels", [P, NCHUNK], mybir.dt.int32, isOutput=False
    )
    cen_d = nc.declare_dram_parameter("centers", [C, D], f32, isOutput=False)
    out_d = nc.declare_dram_parameter("out", [P, NCHUNK], f32, isOutput=True)

    x_r = x_d.rearrange("(p r) d -> p r d", p=P)  # [128, 16, 512], contiguous per p

    with ExitStack() as ctx:
        x_sb = ctx.enter_context(nc.sbuf_tensor("x_sb", [P, NCHUNK, D], f32))
        g_sb = ctx.enter_context(nc.sbuf_tensor("g_sb", [P, NCHUNK, D], f32))
        diff_sb = ctx.enter_context(nc.sbuf_tensor("diff_sb", [P, 2, D], f32))
        sq_sb = ctx.enter_context(nc.sbuf_tensor("sq_sb", [P, D], f32))
        lbl_sb = ctx.enter_context(nc.sbuf_tensor("lbl_sb", [P, NCHUNK], mybir.dt.int32))
        acc_sb = ctx.enter_context(nc.sbuf_tensor("acc_sb", [P, NCHUNK], f32))

        block = ctx.enter_context(nc.Block())
        ls = ctx.enter_context(nc.semaphore("ls"))
        xs = [ctx.enter_context(nc.semaphore(f"xs{q}")) for q in range(NXD)]
        gs = [ctx.enter_context(nc.semaphore(f"gs{r}")) for r in range(NCHUNK)]
        vs = ctx.enter_context(nc.semaphore("vs"))
        ss = ctx.enter_context(nc.semaphore("ss"))
        os_ = ctx.enter_context(nc.semaphore("os"))

        CPX = NCHUNK // NXD  # chunks per x load

        @block.sync
        def _(sync):
            # labels first: the gather stream (Q7 descriptor emission) is the
            # critical path and only needs this tiny tile
            sync.dma_start(out=lbl_sb[:], in_=lbl_d[:]).then_inc(ls, 16)
            for q in range(NXD):
                sync.dma_start(
                    out=x_sb[:, q * CPX : (q + 1) * CPX, :],
                    in_=x_r[:, q * CPX : (q + 1) * CPX, :],
                ).then_inc(xs[q], 16)


        @block.gpsimd
        def _(gpsimd):
            gpsimd.wait_ge(ls, 16)
            for r in range(NCHUNK):
                gpsimd.indirect_dma_start(
                    out=g_sb[:, r, :],
                    out_offset=None,
                    in_=cen_d[:],
                    in_offset=bass.IndirectOffsetOnAxis(
                        ap=lbl_sb[:, r : r + 1], axis=0
                    ),
                ).then_inc(gs[r], 16)

        @block.vector
        def _(vector):
            for r in range(NCHUNK):
                vector.wait_ge(xs[r // CPX], 16)
                vector.wait_ge(gs[r], 16)
                if r >= 2:
                    vector.wait_ge(ss, r - 1)  # WAR: scalar done with diff slot
                vector.tensor_tensor(
                    out=diff_sb[:, r % 2, :],
                    in0=x_sb[:, r, :],
                    in1=g_sb[:, r, :],
                    op=mybir.AluOpType.subtract,
                ).then_inc(vs, 1)

        @block.scalar
        def _(scalar):
            for r in range(NCHUNK):
                scalar.wait_ge(vs, r + 1)
                scalar.activation(
                    out=sq_sb[:, :],
                    in_=diff_sb[:, r % 2, :],
                    func=mybir.ActivationFunctionType.Square,
                    accum_out=acc_sb[:, r : r + 1],
                ).then_inc(ss, 1)
                if r == NCHUNK - 3:
                    # early store of the first 14 columns hides most of the
                    # final DMA's completion receipt behind the last chunks
                    scalar.dma_start(
                        out=out_d[:, : NCHUNK - 2], in_=acc_sb[:, : NCHUNK - 2]
                    ).then_inc(os_, 16)
            scalar.dma_start(
                out=out_d[:, NCHUNK - 2 :], in_=acc_sb[:, NCHUNK - 2 :]
            ).then_inc(os_, 16)
            scalar.wait_ge(os_, 32)

    nc.finalize()
    return nc


def _get_nc():
    if "nc" not in _NC_CACHE:
        _NC_CACHE["nc"] = build_nc()
    return _NC_CACHE["nc"]


def kernel(x, labels, centers, _trace=False):
    x = np.asarray(x, dtype=np.float32)
    centers = np.asarray(centers, dtype=np.float32)
    labels_i = np.asarray(labels).astype(np.int32)

    in_maps = []
    for i in range(N_CORES):
        xs_ = np.ascontiguousarray(x[i * B_SHARD : (i + 1) * B_SHARD])
        ls_ = labels_i[i * B_SHARD : (i + 1) * B_SHARD]
        in_maps.append(
            {
                "x": xs_,
                # row 16p + r at [p, r]
                "labels": np.ascontiguousarray(ls_.reshape(P, NCHUNK)),
                "centers": centers,
            }
        )

    nc = _get_nc()
    res = run_bass_kernel_spmd(nc, in_maps, list(range(N_CORES)), trace=_trace)
    partials = np.stack([r["out"] for r in res.results])  # [8, 128, 16]
    total = np.sum(partials.astype(np.float64))
    total += B * (C - 1) * CLAMP_MIN
    loss = np.float32(total / B)
    if _trace:
        return np.asarray(loss), res
    return np.asarray(loss)


# revision 29
# speedup vs baseline: 1.2547x; 1.0437x over previous
"""CenterLoss kernel for Trainium2, data-parallel across 8 NeuronCores.

Math: the reference masks the full [B, C] squared-distance matrix with
one_hot(labels) and clamps to [1e-12, 1e12] before summing.  The mask keeps
only distmat[i, labels[i]]; every other entry becomes clip(0) = 1e-12.  The
kept entries are ~1024 (chi-square-like, 512 dof), so the clamp never binds
on them and the loss reduces to

    loss = ( sum_i ||x_i - c_{l_i}||^2 + B*(C-1)*1e-12 ) / B

Per core (B/8 = 2048 rows), raw bass pipeline, p-major row layout
(shard row 16*p + r lives at partition p, chunk r; r in [0,16)):
  sync   : 4x 1MB x loads (HWDGE, 8KB/partition contiguous descriptors)
  gpsimd : label load + 16x indirect_dma_start (row r: out[p,:] =
           centers[labels[16p+r], :])
  vector : diff = x - g per chunk            [128, 512]
  scalar : Square activation + row-accumulate -> acc[:, r], then acc store
Host sums the 8 x [128, 16] partials in f64 and adds the clamp constant.
"""

import sys
from contextlib import ExitStack

import numpy as np

try:
    import concourse.bass  # noqa: F401
except ImportError:
    sys.path.insert(0, "/opt/trn_rl_repo")

import concourse.bass as bass
import concourse.mybir as mybir
from concourse.bacc import Bacc
from concourse.bass_utils import run_bass_kernel_spmd

B, C, D = 16384, 1000, 512
N_CORES = 8
B_SHARD = B // N_CORES  # 2048
P = 128
NCHUNK = B_SHARD // P  # 16 chunks, chunk r = rows {16p + r}
NXD = 4  # x loads (4 chunks each)
CLAMP_MIN = 1e-12
CLAMP_MAX = 1e12

_NC_CACHE = {}


def build_nc():
    nc = Bacc()
    f32 = mybir.dt.float32
    x_d = nc.declare_dram_parameter("x", [B_SHARD, D], f32, isOutput=False)
    lbl_d = nc.declare_dram_parameter(
        "labels", [P, NCHUNK], mybir.dt.int32, isOutput=False
    )
    cen_d = nc.declare_dram_parameter("centers", [C, D], f32, isOutput=False)
    out_d = nc.declare_dram_parameter("out", [P, NCHUNK], f32, isOutput=True)

    x_r = x_d.rearrange("(p r) d -> p r d", p=P)  # [128, 16, 512], contiguous per p

    with ExitStack() as ctx:
        x_sb = ctx.enter_context(nc.sbuf_tensor("x_sb", [P, NCHUNK, D], f32))
        g_sb = ctx.enter_context(nc.sbuf_tensor("g_sb", [P, NCHUNK, D], f32))
        # diff/sq live in PSUM: relieves SBUF write-port pressure that
        # contends with Q7's SWDGE descriptor rings during gather emission
        diff_ps = [
            ctx.enter_context(nc.psum_tensor(f"diff_ps{i}", [P, D], f32))
            for i in range(2)
        ]
        sq_ps = ctx.enter_context(nc.psum_tensor("sq_ps", [P, D], f32))
        lbl_sb = ctx.enter_context(nc.sbuf_tensor("lbl_sb", [P, NCHUNK], mybir.dt.int32))
        acc_sb = ctx.enter_context(nc.sbuf_tensor("acc_sb", [P, NCHUNK], f32))

        block = ctx.enter_context(nc.Block())
        ls = ctx.enter_context(nc.semaphore("ls"))
        xs = [ctx.enter_context(nc.semaphore(f"xs{q}")) for q in range(NXD)]
        gs = [ctx.enter_context(nc.semaphore(f"gs{r}")) for r in range(NCHUNK)]
        vs = ctx.enter_context(nc.semaphore("vs"))
        ss = ctx.enter_context(nc.semaphore("ss"))
        os_ = ctx.enter_context(nc.semaphore("os"))

        CPX = NCHUNK // NXD  # chunks per x load

        @block.sync
        def _(sync):
            # labels first: the gather stream (Q7 descriptor emission) is the
            # critical path and only needs this tiny tile
            sync.dma_start(out=lbl_sb[:], in_=lbl_d[:]).then_inc(ls, 16)
            for q in range(NXD):
                sync.dma_start(
                    out=x_sb[:, q * CPX : (q + 1) * CPX, :],
                    in_=x_r[:, q * CPX : (q + 1) * CPX, :],
                ).then_inc(xs[q], 16)


        @block.gpsimd
        def _(gpsimd):
            gpsimd.wait_ge(ls, 16)
            for r in range(NCHUNK):
                gpsimd.indirect_dma_start(
                    out=g_sb[:, r, :],
                    out_offset=None,
                    in_=cen_d[:],
                    in_offset=bass.IndirectOffsetOnAxis(
                        ap=lbl_sb[:, r : r + 1], axis=0
                    ),
                ).then_inc(gs[r], 16)

        @block.vector
        def _(vector):
            for r in range(NCHUNK):
                vector.wait_ge(xs[r // CPX], 16)
                vector.wait_ge(gs[r], 16)
                if r >= 2:
                    vector.wait_ge(ss, r - 1)  # WAR: scalar done with diff slot
                vector.tensor_tensor(
                    out=diff_ps[r % 2][:, :],
                    in0=x_sb[:, r, :],
                    in1=g_sb[:, r, :],
                    op=mybir.AluOpType.subtract,
                ).then_inc(vs, 1)

        @block.scalar
        def _(scalar):
            for r in range(NCHUNK):
                scalar.wait_ge(vs, r + 1)
                scalar.activation(
                    out=sq_ps[:, :],
                    in_=diff_ps[r % 2][:, :],
                    func=mybir.ActivationFunctionType.Square,
                    accum_out=acc_sb[:, r : r + 1],
                ).then_inc(ss, 1)
                if r == NCHUNK - 3:
                    # early store of the first 14 columns hides most of the
                    # final DMA's completion receipt behind the last chunks
                    scalar.dma_start(
                        out=out_d[:, : NCHUNK - 2], in_=acc_sb[:, : NCHUNK - 2]
                    ).then_inc(os_, 16)
            scalar.dma_start(
                out=out_d[:, NCHUNK - 2 :], in_=acc_sb[:, NCHUNK - 2 :]
            ).then_inc(os_, 16)
            scalar.wait_ge(os_, 32)

    nc.finalize()
    return nc


def _get_nc():
    if "nc" not in _NC_CACHE:
        _NC_CACHE["nc"] = build_nc()
    return _NC_CACHE["nc"]


def kernel(x, labels, centers, _trace=False):
    x = np.asarray(x, dtype=np.float32)
    centers = np.asarray(centers, dtype=np.float32)
    labels_i = np.asarray(labels).astype(np.int32)

    in_maps = []
    for i in range(N_CORES):
        xs_ = np.ascontiguousarray(x[i * B_SHARD : (i + 1) * B_SHARD])
        ls_ = labels_i[i * B_SHARD : (i + 1) * B_SHARD]
        in_maps.append(
            {
                "x": xs_,
                # row 16p + r at [p, r]
                "labels": np.ascontiguousarray(ls_.reshape(P, NCHUNK)),
                "centers": centers,
            }
        )

    nc = _get_nc()
    res = run_bass_kernel_spmd(nc, in_maps, list(range(N_CORES)), trace=_trace)
    partials = np.stack([r["out"] for r in res.results])  # [8, 128, 16]
    total = np.sum(partials.astype(np.float64))
    total += B * (C - 1) * CLAMP_MIN
    loss = np.float32(total / B)
    if _trace:
        return np.asarray(loss), res
    return np.asarray(loss)
